# revision 8
# baseline (speedup 1.0000x reference)
"""CAM (channel attention) module kernel for Trainium2, 8 NeuronCores.

Reference computation (per sample, x: [C, N] with C=512, N=64*64):
    energy    = x @ x.T                      # [C, C] symmetric Gram matrix
    energy_n  = rowmax(energy) - energy
    att       = softmax(energy_n, axis=-1)
    out       = gamma * (att @ x) + x

Softmax shift-invariance: softmax(rowmax - e) == softmax(-e), stabilized
with the row-min m_i:  att[i,j] = exp(m_i - e_ij) / S_i,  S_i = sum_j.

Sharding: pure data parallel over batch B=16 -> 2 samples per core.

Precision: the Gram matrix is computed from fp16 operands (10 mantissa
bits; ~1e-2 relative vs float64 at gamma=1), mm2 runs in fp32r; all
accumulation is fp32 PSUM and the "+ x" epilogue is exact fp32, so
gamma=0 reproduces x bit-exactly. fp16 runs the PE at 1 cycle/row for
ANY moving width (fp32r pays 4x below 256 wide), which makes the exact
128-block triangle + 128-wide PT matmuls cheap. The natural-layout
tiles are DECLARED f32r (bit-identical to f32), so no rounding pass is
needed anywhere: f32 -> fp16 rounding happens for free inside the
transpose-PSUM evacuations, and the PE truncates f32r mantissas itself.

Per-core pipeline (2 samples):
  1. load xf natural in 8 interleaved column pieces on the sync queue
     (output stores go on the otherwise-idle GpSimd queue so the next
     sample's loads are never stuck behind them); warmup matmuls keep
     the PE clock un-throttled (HAM) while the first pieces land
  2. per 128-col chunk k: PE-transpose the 4 channel blocks (f32r,
     1.5 cyc/row) -> PSUM, evacuate to fp16 xt (alternating ACT/DVE),
     then advance ALL four triangular Gram panels by one k step
     (wavefront) so the PE never queues idle work behind DMA pacing
  3. energy is symmetric: row panel ci computes columns [128*ci : 512]
     (exact upper triangle); lower blocks are mirrored from finished
     panels via PE transposes of stashed SBUF copies
  4. software-pipelined tail: for each ci - softmax(ci) [rowmin on DVE,
     P16 = exp(m - e) with fused row-sum on ACT, D = diag(gamma/S) in
     one fused DVE tensor_scalar], then mm2 block ci-1, then
     PT(ci) = P[ci].T @ D[ci] (4 fp16 128-wide matmuls, evacuated f32r
     in bj-grouped layout). mm2 for output block ci only needs
     softmax(ci), so each softmax hides under the previous mm2 block;
     only softmax(0)'s ~2.5us latency is exposed per sample
  5. mm2 block ci: out[ci] = sum_bj PT_ci[bj].T @ nat[bj] (moving
     operand is the f32r nat - no copies), epilogue out = psum + x in
     one DVE scalar_tensor_tensor pass (x read as exact fp32 bits)
  6. the next sample's FULL load is emitted before mm2 so it streams in
     during the ~14us of mm2 blocks and the boundary never stalls
"""

import numpy as np

import concourse.bacc as bacc
import concourse.tile as tile
from concourse import mybir
from concourse.bass_utils import run_bass_kernel_spmd
from concourse.masks import make_identity

B, C, H, W = 16, 512, 64, 64
N = H * W
NCORES = 8
BPC = B // NCORES  # samples per core
CB = C // 128      # channel blocks (4)
NK = N // 128      # 128-wide n-chunks (32)
NT = N // 512      # 512-wide n-tiles (8)

F32 = mybir.dt.float32
F32R = mybir.dt.float32r
F16 = mybir.dt.float16


def _emit(nc, tc, ctx, x, gamma, out):
    consts = ctx.enter_context(tc.tile_pool(name="consts", bufs=1))
    nat_pool = ctx.enter_context(tc.tile_pool(name="nat", bufs=2 * CB))
    xfT_pool = ctx.enter_context(tc.tile_pool(name="xfT", bufs=NK))
    p_pool = ctx.enter_context(tc.tile_pool(name="p", bufs=CB))
    pt_pool = ctx.enter_context(tc.tile_pool(name="pt", bufs=CB))
    eblk_pool = ctx.enter_context(tc.tile_pool(name="eblk", bufs=6))
    d_pool = ctx.enter_context(tc.tile_pool(name="d", bufs=CB))
    small = ctx.enter_context(tc.tile_pool(name="small", bufs=4 * CB))
    outs_pool = ctx.enter_context(tc.tile_pool(name="outs", bufs=4))
    psum_e = ctx.enter_context(tc.tile_pool(name="psum_e", bufs=CB, space="PSUM"))
    psum_g = ctx.enter_context(tc.tile_pool(name="psum_g", bufs=4, space="PSUM"))

    identity = consts.tile([128, 128], F32)
    make_identity(nc, identity[:])
    idR_t = consts.tile([128, 128], F32R)
    nc.vector.tensor_copy(out=idR_t[:], in_=identity[:])
    idR = idR_t[:]
    id16 = consts.tile([128, 128], F16)
    nc.vector.tensor_copy(out=id16[:], in_=identity[:])
    g_sb = consts.tile([128, 1], F32)
    nc.gpsimd.dma_start(out=g_sb[:], in_=gamma[:].to_broadcast((128, 1)))

    QN = N // 8

    def load_sample(s):
        nat = [
            nat_pool.tile([128, N], F32R, tag="nat", name=f"nat{s}_{c}")
            for c in range(CB)
        ]
        for q in range(8):
            for c in range(CB):
                nc.sync.dma_start(
                    out=nat[c][:, QN * q : QN * (q + 1)],
                    in_=x[
                        s, 128 * c : 128 * (c + 1), QN * q : QN * (q + 1)
                    ].bitcast(F32R),
                )
        return nat

    nats = {0: load_sample(0)}
    for s in range(BPC):
        nat = nats.pop(s)

        if s == 0:
            # keep the PE busy (HAM warm) while the first load pieces land
            warm_ps = psum_g.tile([128, 128], F32, tag="g", name=f"warm{s}")
            for w in range(28):
                nc.tensor.matmul(warm_ps[:], id16[:], id16[:], start=(w == 0), stop=False)
            nc.tensor.matmul(warm_ps[:], id16[:], id16[:], start=False, stop=True)

        # ---- transpose + triangular Gram wavefront: per k-chunk, PE
        # transposes the 4 blocks (f32r), the chunk is evacuated to fp16,
        # then ALL four row panels advance one accumulation step ----
        # panel ci covers columns [128*ci : 512] (upper triangle incl diag)
        e_ps = [
            psum_e.tile([128, C], F32, tag="e", name=f"e_ps{s}_{ci}")
            for ci in range(CB)
        ]
        for k in range(NK):
            t_ps = psum_g.tile([128, C], F32R, tag="g")
            for c in range(CB):
                nc.tensor.transpose(
                    t_ps[:, 128 * c : 128 * (c + 1)],
                    nat[c][:, 128 * k : 128 * (k + 1)],
                    idR,
                )
            xt = xfT_pool.tile([128, C], F16, tag="xfT")
            if k % 2 == 0:
                nc.scalar.activation(
                    out=xt[:], in_=t_ps[:].bitcast(F32),
                    func=mybir.ActivationFunctionType.Copy,
                    bias=0.0, scale=1.0,
                )
            else:
                nc.vector.tensor_copy(out=xt[:], in_=t_ps[:].bitcast(F32))
            for ci in range(CB):
                lo = 128 * ci
                nc.tensor.matmul(
                    e_ps[ci][:, lo:C],
                    xt[:, lo : lo + 128],
                    xt[:, lo:C],
                    start=(k == 0),
                    stop=(k == NK - 1),
                )

        # ---- software-pipelined softmax(ci) / mm2(ci-1) / PT(ci) ----
        e_blk = {}  # (ci, cj) -> SBUF f32 copy of energy block for mirroring
        pt_c = []   # per ci: PT columns 128*ci..128*(ci+1), bj-grouped

        def mm2_block(ci, mid=None):
            # out[ci] = sum_bj PT_ci[bj].T @ nat[bj] ; epilogue += x
            # nt-pairs reuse the stationary operand (1 LDWEIGHTS / 2 matmuls)
            ptc = pt_c[ci]
            for ntg in range(NT // 2):
                if mid is not None and ntg == NT // 2 - 1:
                    mid()
                nts = (2 * ntg, 2 * ntg + 1)
                ops = [
                    psum_g.tile([128, 512], F32, tag="g", name=f"ops{s}_{ci}_{nt}")
                    for nt in nts
                ]
                for bj in range(CB):
                    for oi, nt in enumerate(nts):
                        nc.tensor.matmul(
                            ops[oi][:],
                            ptc[:, 128 * bj : 128 * (bj + 1)],
                            nat[bj][:, 512 * nt : 512 * (nt + 1)],
                            start=(bj == 0),
                            stop=(bj == CB - 1),
                        )
                for oi, nt in enumerate(nts):
                    o_sb = outs_pool.tile([128, 512], F32, tag="o")
                    nc.vector.scalar_tensor_tensor(
                        out=o_sb[:],
                        in0=ops[oi][:],
                        scalar=1.0,
                        in1=nat[ci][:, 512 * nt : 512 * (nt + 1)].bitcast(F32),
                        op0=mybir.AluOpType.bypass,
                        op1=mybir.AluOpType.add,
                    )
                    nc.gpsimd.dma_start(
                        out=out[
                            s, 128 * ci : 128 * (ci + 1), 512 * nt : 512 * (nt + 1)
                        ],
                        in_=o_sb[:],
                    )

        for ci in range(CB):
            e = e_ps[ci]
            # stash SBUF copies of the blocks later row-panels will mirror
            for cj in range(ci + 1, CB):
                blk = eblk_pool.tile(
                    [128, 128], F32, tag="eblk", name=f"eblk{s}_{ci}_{cj}"
                )
                nc.vector.tensor_copy(
                    out=blk[:], in_=e[:, 128 * cj : 128 * (cj + 1)]
                )
                e_blk[(ci, cj)] = blk
            # mirror missing lower blocks from earlier panels
            for cj in range(ci):
                nc.tensor.transpose(
                    e[:, 128 * cj : 128 * (cj + 1)],
                    e_blk[(cj, ci)][:],
                    identity[:],
                )
            # softmax pieces: P = exp(m - e), S = rowsum, D = diag(gamma/S)
            m = small.tile([128, 1], F32, tag="m")
            nc.vector.tensor_reduce(
                out=m[:], in_=e[:], axis=mybir.AxisListType.X,
                op=mybir.AluOpType.min,
            )
            p = p_pool.tile([128, C], F16, tag="p")
            ssum = small.tile([128, 1], F32, tag="s")
            nc.scalar.activation(
                out=p[:], in_=e[:],
                func=mybir.ActivationFunctionType.Exp,
                bias=m[:], scale=-1.0, accum_out=ssum[:],
            )
            r = small.tile([128, 1], F32, tag="r")
            nc.vector.reciprocal(out=r[:], in_=ssum[:])
            d = d_pool.tile([128, 128], F16, tag="d")
            nc.vector.tensor_scalar(
                out=d[:], in0=identity[:], scalar1=r[:], scalar2=g_sb[:],
                op0=mybir.AluOpType.mult, op1=mybir.AluOpType.mult,
            )

            # PT(ci) = P[ci].T @ D[ci]: [j, i] = gamma * att[i, j] for
            # i in block ci, laid out bj-grouped along the free axis.
            # Emitted near the end of mm2_block(ci-1) so the PSUM->SBUF
            # evacuation hides under the previous block's last matmuls.
            def emit_pt(p=p, d=d, ci=ci):
                ptp = psum_g.tile([128, C], F32, tag="g", name=f"ptp{s}_{ci}")
                for bj in range(CB):
                    nc.tensor.matmul(
                        ptp[:, 128 * bj : 128 * (bj + 1)],
                        p[:, 128 * bj : 128 * (bj + 1)],
                        d[:],
                        start=True,
                        stop=True,
                    )
                ptc = pt_pool.tile([128, C], F32R, tag="pt", name=f"ptc{s}_{ci}")
                nc.scalar.activation(
                    out=ptc[:], in_=ptp[:],
                    func=mybir.ActivationFunctionType.Copy,
                    bias=0.0, scale=1.0,
                )
                pt_c.append(ptc)

            # overlap: previous output block's mm2 runs while this
            # panel's softmax is still on DVE/ACT
            if ci > 0:
                mm2_block(ci - 1, mid=emit_pt)
            else:
                emit_pt()
                if s + 1 < BPC:
                    # next sample's load streams in during the mm2 blocks
                    nats[s + 1] = load_sample(s + 1)

        mm2_block(CB - 1)


_NC_CACHE = None


def _build():
    global _NC_CACHE
    if _NC_CACHE is not None:
        return _NC_CACHE
    from contextlib import ExitStack

    nc = bacc.Bacc("TRN2", target_bir_lowering=False)
    x = nc.dram_tensor("x", [BPC, C, N], F32, kind="ExternalInput")
    gamma = nc.dram_tensor("gamma", [1, 1], F32, kind="ExternalInput")
    out = nc.dram_tensor("out", [BPC, C, N], F32, kind="ExternalOutput")
    with tile.TileContext(nc) as tc:
        with ExitStack() as ctx:
            _emit(nc, tc, ctx, x[:], gamma[:], out[:])
    nc.compile()
    _NC_CACHE = nc
    return nc


def kernel(x, gamma):
    x = np.ascontiguousarray(np.asarray(x, dtype=np.float32))
    gamma = np.ascontiguousarray(np.asarray(gamma, dtype=np.float32))
    assert x.shape == (B, C, H, W), x.shape
    xf = x.reshape(B, C, N)
    nc = _build()
    in_maps = [
        {
            "x": xf[c * BPC : (c + 1) * BPC],
            "gamma": gamma.reshape(1, 1),
        }
        for c in range(NCORES)
    ]
    res = run_bass_kernel_spmd(nc, in_maps, core_ids=list(range(NCORES)))
    out = np.concatenate([res.results[c]["out"] for c in range(NCORES)], axis=0)
    return out.reshape(B, C, H, W)


# revision 9
# speedup vs baseline: 1.0053x; 1.0053x over previous
"""CAM (channel attention) module kernel for Trainium2, 8 NeuronCores.

Reference computation (per sample, x: [C, N] with C=512, N=64*64):
    energy    = x @ x.T                      # [C, C] symmetric Gram matrix
    energy_n  = rowmax(energy) - energy
    att       = softmax(energy_n, axis=-1)
    out       = gamma * (att @ x) + x

Softmax shift-invariance: softmax(rowmax - e) == softmax(-e), stabilized
with the row-min m_i:  att[i,j] = exp(m_i - e_ij) / S_i,  S_i = sum_j.

Sharding: pure data parallel over batch B=16 -> 2 samples per core.

Precision: the Gram matrix is computed from fp16 operands (10 mantissa
bits; ~1e-2 relative vs float64 at gamma=1), mm2 runs in fp32r; all
accumulation is fp32 PSUM and the "+ x" epilogue is exact fp32, so
gamma=0 reproduces x bit-exactly. fp16 runs the PE at 1 cycle/row for
ANY moving width (fp32r pays 4x below 256 wide), which makes the exact
128-block triangle + 128-wide PT matmuls cheap. The natural-layout
tiles are DECLARED f32r (bit-identical to f32), so no rounding pass is
needed anywhere: f32 -> fp16 rounding happens for free inside the
transpose-PSUM evacuations, and the PE truncates f32r mantissas itself.

Per-core pipeline (2 samples):
  1. load xf natural in 8 interleaved column pieces on the sync queue
     (output stores go on the otherwise-idle GpSimd queue so the next
     sample's loads are never stuck behind them); warmup matmuls keep
     the PE clock un-throttled (HAM) while the first pieces land
  2. per 128-col chunk k: PE-transpose the 4 channel blocks (f32r,
     1.5 cyc/row) -> PSUM, evacuate to fp16 xt (alternating ACT/DVE),
     then advance ALL four triangular Gram panels by one k step
     (wavefront) so the PE never queues idle work behind DMA pacing
  3. energy is symmetric: row panel ci computes columns [128*ci : 512]
     (exact upper triangle); lower blocks are mirrored from finished
     panels via PE transposes of stashed SBUF copies
  4. software-pipelined tail: for each ci - softmax(ci) [rowmin on DVE,
     P16 = exp(m - e) with fused row-sum on ACT, D = diag(gamma/S) in
     one fused DVE tensor_scalar], then mm2 block ci-1, then
     PT(ci) = P[ci].T @ D[ci] (4 fp16 128-wide matmuls, evacuated f32r
     in bj-grouped layout). mm2 for output block ci only needs
     softmax(ci), so each softmax hides under the previous mm2 block;
     only softmax(0)'s ~2.5us latency is exposed per sample
  5. mm2 block ci: out[ci] = sum_bj PT_ci[bj].T @ nat[bj] (moving
     operand is the f32r nat - no copies), epilogue out = psum + x in
     one DVE scalar_tensor_tensor pass (x read as exact fp32 bits)
  6. the next sample's FULL load is emitted before mm2 so it streams in
     during the ~14us of mm2 blocks and the boundary never stalls
"""

import numpy as np

import concourse.bacc as bacc
import concourse.tile as tile
from concourse import mybir
from concourse.bass_utils import run_bass_kernel_spmd
from concourse.masks import make_identity

B, C, H, W = 16, 512, 64, 64
N = H * W
NCORES = 8
BPC = B // NCORES  # samples per core
CB = C // 128      # channel blocks (4)
NK = N // 128      # 128-wide n-chunks (32)
NT = N // 512      # 512-wide n-tiles (8)

F32 = mybir.dt.float32
F32R = mybir.dt.float32r
F16 = mybir.dt.float16


def _emit(nc, tc, ctx, x, gamma, out):
    consts = ctx.enter_context(tc.tile_pool(name="consts", bufs=1))
    nat_pool = ctx.enter_context(tc.tile_pool(name="nat", bufs=2 * CB))
    xfT_pool = ctx.enter_context(tc.tile_pool(name="xfT", bufs=NK))
    p_pool = ctx.enter_context(tc.tile_pool(name="p", bufs=CB))
    pt_pool = ctx.enter_context(tc.tile_pool(name="pt", bufs=CB))
    eblk_pool = ctx.enter_context(tc.tile_pool(name="eblk", bufs=6))
    d_pool = ctx.enter_context(tc.tile_pool(name="d", bufs=CB))
    small = ctx.enter_context(tc.tile_pool(name="small", bufs=4 * CB))
    outs_pool = ctx.enter_context(tc.tile_pool(name="outs", bufs=4))
    psum_e = ctx.enter_context(tc.tile_pool(name="psum_e", bufs=CB, space="PSUM"))
    psum_g = ctx.enter_context(tc.tile_pool(name="psum_g", bufs=4, space="PSUM"))

    identity = consts.tile([128, 128], F32)
    make_identity(nc, identity[:])
    idR_t = consts.tile([128, 128], F32R)
    nc.vector.tensor_copy(out=idR_t[:], in_=identity[:])
    idR = idR_t[:]
    id16 = consts.tile([128, 128], F16)
    nc.vector.tensor_copy(out=id16[:], in_=identity[:])
    g_sb = consts.tile([128, 1], F32)
    nc.gpsimd.dma_start(out=g_sb[:], in_=gamma[:].to_broadcast((128, 1)))

    QN = N // 8

    def load_sample(s):
        nat = [
            nat_pool.tile([128, N], F32R, tag="nat", name=f"nat{s}_{c}")
            for c in range(CB)
        ]
        for q in range(8):
            for c in range(CB):
                nc.sync.dma_start(
                    out=nat[c][:, QN * q : QN * (q + 1)],
                    in_=x[
                        s, 128 * c : 128 * (c + 1), QN * q : QN * (q + 1)
                    ].bitcast(F32R),
                )
        return nat

    nats = {0: load_sample(0)}
    for s in range(BPC):
        nat = nats.pop(s)

        if s == 0:
            # keep the PE busy (HAM warm) while the first load pieces land
            warm_ps = psum_g.tile([128, 128], F32, tag="g", name=f"warm{s}")
            for w in range(28):
                nc.tensor.matmul(warm_ps[:], id16[:], id16[:], start=(w == 0), stop=False)
            nc.tensor.matmul(warm_ps[:], id16[:], id16[:], start=False, stop=True)

        # ---- transpose + triangular Gram wavefront: per k-chunk, PE
        # transposes the 4 blocks (f32r), the chunk is evacuated to fp16,
        # then ALL four row panels advance one accumulation step ----
        # panel ci covers columns [128*ci : 512] (upper triangle incl diag)
        e_ps = [
            psum_e.tile([128, C], F32, tag="e", name=f"e_ps{s}_{ci}")
            for ci in range(CB)
        ]
        for k in range(NK):
            t_ps = psum_g.tile([128, C], F32R, tag="g")
            for c in range(CB):
                nc.tensor.transpose(
                    t_ps[:, 128 * c : 128 * (c + 1)],
                    nat[c][:, 128 * k : 128 * (k + 1)],
                    idR,
                )
            xt = xfT_pool.tile([128, C], F16, tag="xfT")
            if k % 2 == 0:
                nc.scalar.activation(
                    out=xt[:], in_=t_ps[:].bitcast(F32),
                    func=mybir.ActivationFunctionType.Copy,
                    bias=0.0, scale=1.0,
                )
            else:
                nc.vector.tensor_copy(out=xt[:], in_=t_ps[:].bitcast(F32))
            for ci in range(CB):
                lo = 128 * ci
                nc.tensor.matmul(
                    e_ps[ci][:, lo:C],
                    xt[:, lo : lo + 128],
                    xt[:, lo:C],
                    start=(k == 0),
                    stop=(k == NK - 1),
                )

        # ---- software-pipelined softmax(ci) / mm2(ci-1) / PT(ci) ----
        e_blk = {}  # (ci, cj) -> SBUF f32 copy of energy block for mirroring
        pt_c = []   # per ci: PT columns 128*ci..128*(ci+1), bj-grouped

        def mm2_block(ci, mid=None):
            # out[ci] = sum_bj PT_ci[bj].T @ nat[bj] ; epilogue += x
            ptc = pt_c[ci]
            for nt in range(NT):
                if mid is not None and nt == NT - 2:
                    mid()
                ops = psum_g.tile([128, 512], F32, tag="g")
                for bj in range(CB):
                    nc.tensor.matmul(
                        ops[:],
                        ptc[:, 128 * bj : 128 * (bj + 1)],
                        nat[bj][:, 512 * nt : 512 * (nt + 1)],
                        start=(bj == 0),
                        stop=(bj == CB - 1),
                    )
                o_sb = outs_pool.tile([128, 512], F32, tag="o")
                nc.vector.scalar_tensor_tensor(
                    out=o_sb[:],
                    in0=ops[:],
                    scalar=1.0,
                    in1=nat[ci][:, 512 * nt : 512 * (nt + 1)].bitcast(F32),
                    op0=mybir.AluOpType.bypass,
                    op1=mybir.AluOpType.add,
                )
                nc.gpsimd.dma_start(
                    out=out[
                        s, 128 * ci : 128 * (ci + 1), 512 * nt : 512 * (nt + 1)
                    ],
                    in_=o_sb[:],
                )

        for ci in range(CB):
            e = e_ps[ci]
            # stash SBUF copies of the blocks later row-panels will mirror
            for cj in range(ci + 1, CB):
                blk = eblk_pool.tile(
                    [128, 128], F32, tag="eblk", name=f"eblk{s}_{ci}_{cj}"
                )
                nc.vector.tensor_copy(
                    out=blk[:], in_=e[:, 128 * cj : 128 * (cj + 1)]
                )
                e_blk[(ci, cj)] = blk
            # mirror missing lower blocks from earlier panels
            for cj in range(ci):
                nc.tensor.transpose(
                    e[:, 128 * cj : 128 * (cj + 1)],
                    e_blk[(cj, ci)][:],
                    identity[:],
                )
            # softmax pieces: P = exp(m - e), S = rowsum, D = diag(gamma/S)
            m = small.tile([128, 1], F32, tag="m")
            nc.vector.tensor_reduce(
                out=m[:], in_=e[:], axis=mybir.AxisListType.X,
                op=mybir.AluOpType.min,
            )
            p = p_pool.tile([128, C], F16, tag="p")
            ssum = small.tile([128, 1], F32, tag="s")
            nc.scalar.activation(
                out=p[:], in_=e[:],
                func=mybir.ActivationFunctionType.Exp,
                bias=m[:], scale=-1.0, accum_out=ssum[:],
            )
            r = small.tile([128, 1], F32, tag="r")
            nc.vector.reciprocal(out=r[:], in_=ssum[:])
            d = d_pool.tile([128, 128], F16, tag="d")
            nc.vector.tensor_scalar(
                out=d[:], in0=identity[:], scalar1=r[:], scalar2=g_sb[:],
                op0=mybir.AluOpType.mult, op1=mybir.AluOpType.mult,
            )

            # PT(ci) = P[ci].T @ D[ci]: [j, i] = gamma * att[i, j] for
            # i in block ci, laid out bj-grouped along the free axis.
            # Emitted near the end of mm2_block(ci-1) so the PSUM->SBUF
            # evacuation hides under the previous block's last matmuls.
            def emit_pt(p=p, d=d, ci=ci):
                ptp = psum_g.tile([128, C], F32, tag="g", name=f"ptp{s}_{ci}")
                for bj in range(CB):
                    nc.tensor.matmul(
                        ptp[:, 128 * bj : 128 * (bj + 1)],
                        p[:, 128 * bj : 128 * (bj + 1)],
                        d[:],
                        start=True,
                        stop=True,
                    )
                ptc = pt_pool.tile([128, C], F32R, tag="pt", name=f"ptc{s}_{ci}")
                nc.scalar.activation(
                    out=ptc[:], in_=ptp[:],
                    func=mybir.ActivationFunctionType.Copy,
                    bias=0.0, scale=1.0,
                )
                pt_c.append(ptc)

            # overlap: previous output block's mm2 runs while this
            # panel's softmax is still on DVE/ACT
            if ci > 0:
                mm2_block(ci - 1, mid=emit_pt)
            else:
                emit_pt()
                if s + 1 < BPC:
                    # next sample's load streams in during the mm2 blocks
                    nats[s + 1] = load_sample(s + 1)

        mm2_block(CB - 1)


_NC_CACHE = None


def _build():
    global _NC_CACHE
    if _NC_CACHE is not None:
        return _NC_CACHE
    from contextlib import ExitStack

    nc = bacc.Bacc("TRN2", target_bir_lowering=False)
    x = nc.dram_tensor("x", [BPC, C, N], F32, kind="ExternalInput")
    gamma = nc.dram_tensor("gamma", [1, 1], F32, kind="ExternalInput")
    out = nc.dram_tensor("out", [BPC, C, N], F32, kind="ExternalOutput")
    with tile.TileContext(nc) as tc:
        with ExitStack() as ctx:
            _emit(nc, tc, ctx, x[:], gamma[:], out[:])
    nc.compile()
    _NC_CACHE = nc
    return nc


def kernel(x, gamma):
    x = np.ascontiguousarray(np.asarray(x, dtype=np.float32))
    gamma = np.ascontiguousarray(np.asarray(gamma, dtype=np.float32))
    assert x.shape == (B, C, H, W), x.shape
    xf = x.reshape(B, C, N)
    nc = _build()
    in_maps = [
        {
            "x": xf[c * BPC : (c + 1) * BPC],
            "gamma": gamma.reshape(1, 1),
        }
        for c in range(NCORES)
    ]
    res = run_bass_kernel_spmd(nc, in_maps, core_ids=list(range(NCORES)))
    out = np.concatenate([res.results[c]["out"] for c in range(NCORES)], axis=0)
    return out.reshape(B, C, H, W)


# revision 10
# speedup vs baseline: 1.0453x; 1.0398x over previous
"""CAM (channel attention) module kernel for Trainium2, 8 NeuronCores.

Reference computation (per sample, x: [C, N] with C=512, N=64*64):
    energy    = x @ x.T                      # [C, C] symmetric Gram matrix
    energy_n  = rowmax(energy) - energy
    att       = softmax(energy_n, axis=-1)
    out       = gamma * (att @ x) + x

Softmax shift-invariance: softmax(rowmax - e) == softmax(-e), stabilized
with the row-min m_i:  att[i,j] = exp(m_i - e_ij) / S_i,  S_i = sum_j.

Sharding: pure data parallel over batch B=16 -> 2 samples per core.

Precision: the Gram matrix is computed from fp16 operands (10 mantissa
bits; ~1e-2 relative vs float64 at gamma=1), mm2 runs in fp32r; all
accumulation is fp32 PSUM and the "+ x" epilogue is exact fp32, so
gamma=0 reproduces x bit-exactly. fp16 runs the PE at 1 cycle/row for
ANY moving width (fp32r pays 4x below 256 wide), which makes the exact
128-block triangle + 128-wide PT matmuls cheap. The natural-layout
tiles are DECLARED f32r (bit-identical to f32), so no rounding pass is
needed anywhere: f32 -> fp16 rounding happens for free inside the
transpose-PSUM evacuations, and the PE truncates f32r mantissas itself.

Per-core pipeline (2 samples):
  1. load xf natural in 8 interleaved column pieces on the sync queue
     (output stores go on the otherwise-idle GpSimd queue so the next
     sample's loads are never stuck behind them); warmup matmuls keep
     the PE clock un-throttled (HAM) while the first pieces land
  2. per 128-col chunk k: PE-transpose the 4 channel blocks (f32r,
     1.5 cyc/row) -> PSUM, evacuate to fp16 xt (alternating ACT/DVE),
     then advance ALL four triangular Gram panels by one k step
     (wavefront) so the PE never queues idle work behind DMA pacing
  3. energy is symmetric: row panel ci computes columns [128*ci : 512]
     (exact upper triangle); lower blocks are mirrored from finished
     panels via PE transposes of stashed SBUF copies
  4. software-pipelined tail: for each ci - softmax(ci) [rowmin on DVE,
     P16 = exp(m - e) with fused row-sum on ACT, D = diag(gamma/S) in
     one fused DVE tensor_scalar], then mm2 block ci-1, then
     PT(ci) = P[ci].T @ D[ci] (4 fp16 128-wide matmuls, evacuated f32r
     in bj-grouped layout). mm2 for output block ci only needs
     softmax(ci), so each softmax hides under the previous mm2 block;
     only softmax(0)'s ~2.5us latency is exposed per sample
  5. mm2 block ci: out[ci] = sum_bj PT_ci[bj].T @ nat[bj] (moving
     operand is the f32r nat - no copies), epilogue out = psum + x in
     one DVE scalar_tensor_tensor pass (x read as exact fp32 bits)
  6. the next sample's FULL load is emitted before mm2 so it streams in
     during the ~14us of mm2 blocks and the boundary never stalls
"""

import numpy as np

import concourse.bacc as bacc
import concourse.tile as tile
from concourse import mybir
from concourse.bass_utils import run_bass_kernel_spmd
from concourse.masks import make_identity

B, C, H, W = 16, 512, 64, 64
N = H * W
NCORES = 8
BPC = B // NCORES  # samples per core
CB = C // 128      # channel blocks (4)
NK = N // 128      # 128-wide n-chunks (32)
NT = N // 512      # 512-wide n-tiles (8)

F32 = mybir.dt.float32
F32R = mybir.dt.float32r
F16 = mybir.dt.float16


def _emit(nc, tc, ctx, x, gamma, out):
    consts = ctx.enter_context(tc.tile_pool(name="consts", bufs=1))
    nat_pool = ctx.enter_context(tc.tile_pool(name="nat", bufs=2 * CB))
    xfT_pool = ctx.enter_context(tc.tile_pool(name="xfT", bufs=NK))
    p_pool = ctx.enter_context(tc.tile_pool(name="p", bufs=CB))
    pt_pool = ctx.enter_context(tc.tile_pool(name="pt", bufs=CB))
    eblk_pool = ctx.enter_context(tc.tile_pool(name="eblk", bufs=6))
    small = ctx.enter_context(tc.tile_pool(name="small", bufs=4 * CB))
    outs_pool = ctx.enter_context(tc.tile_pool(name="outs", bufs=4))
    psum_e = ctx.enter_context(tc.tile_pool(name="psum_e", bufs=CB, space="PSUM"))
    psum_g = ctx.enter_context(tc.tile_pool(name="psum_g", bufs=4, space="PSUM"))

    identity = consts.tile([128, 128], F32)
    make_identity(nc, identity[:])
    idR_t = consts.tile([128, 128], F32R)
    nc.vector.tensor_copy(out=idR_t[:], in_=identity[:])
    idR = idR_t[:]
    id16 = consts.tile([128, 128], F16)
    nc.vector.tensor_copy(out=id16[:], in_=identity[:])
    g_sb = consts.tile([128, 1], F32)
    nc.gpsimd.dma_start(out=g_sb[:], in_=gamma[:].to_broadcast((128, 1)))

    QN = N // 8

    def load_sample(s):
        nat = [
            nat_pool.tile([128, N], F32R, tag="nat", name=f"nat{s}_{c}")
            for c in range(CB)
        ]
        for q in range(8):
            for c in range(CB):
                nc.sync.dma_start(
                    out=nat[c][:, QN * q : QN * (q + 1)],
                    in_=x[
                        s, 128 * c : 128 * (c + 1), QN * q : QN * (q + 1)
                    ].bitcast(F32R),
                )
        return nat

    nats = {0: load_sample(0)}
    for s in range(BPC):
        nat = nats.pop(s)

        if s == 0:
            # keep the PE busy (HAM warm) while the first load pieces land
            warm_ps = psum_g.tile([128, 128], F32, tag="g", name=f"warm{s}")
            for w in range(28):
                nc.tensor.matmul(warm_ps[:], id16[:], id16[:], start=(w == 0), stop=False)
            nc.tensor.matmul(warm_ps[:], id16[:], id16[:], start=False, stop=True)

        # ---- transpose + triangular Gram wavefront: per k-chunk, PE
        # transposes the 4 blocks (f32r), the chunk is evacuated to fp16,
        # then ALL four row panels advance one accumulation step ----
        # panel ci covers columns [128*ci : 512] (upper triangle incl diag)
        e_ps = [
            psum_e.tile([128, C], F32, tag="e", name=f"e_ps{s}_{ci}")
            for ci in range(CB)
        ]
        # superchunks: transpose 8 chunks as one burst, then run their 32
        # panel matmuls back-to-back - the dense regular-matmul burst keeps
        # the HAM clock gate open (transpose-mode does not count as PE-busy)
        SCK = 8
        for sc in range(NK // SCK):
            xts = []
            for kk in range(SCK):
                k = SCK * sc + kk
                t_ps = psum_g.tile([128, C], F32R, tag="g")
                for c in range(CB):
                    nc.tensor.transpose(
                        t_ps[:, 128 * c : 128 * (c + 1)],
                        nat[c][:, 128 * k : 128 * (k + 1)],
                        idR,
                    )
                xt = xfT_pool.tile([128, C], F16, tag="xfT")
                if k % 2 == 0:
                    nc.scalar.activation(
                        out=xt[:], in_=t_ps[:].bitcast(F32),
                        func=mybir.ActivationFunctionType.Copy,
                        bias=0.0, scale=1.0,
                    )
                else:
                    nc.vector.tensor_copy(out=xt[:], in_=t_ps[:].bitcast(F32))
                xts.append(xt)
            for kk, xt in enumerate(xts):
                k = SCK * sc + kk
                for ci in range(CB):
                    lo = 128 * ci
                    nc.tensor.matmul(
                        e_ps[ci][:, lo:C],
                        xt[:, lo : lo + 128],
                        xt[:, lo:C],
                        start=(k == 0),
                        stop=(k == NK - 1),
                    )

        # ---- software-pipelined softmax(ci) / mm2(ci-1) / PT(ci) ----
        e_blk = {}  # (ci, cj) -> SBUF f32 copy of energy block for mirroring
        pt_c = []   # per ci: PT columns 128*ci..128*(ci+1), bj-grouped
        gs_c = []   # per ci: gamma/S row scales for the epilogue

        def mm2_block(ci, mid=None):
            # out[ci] = sum_bj PT_ci[bj].T @ nat[bj] ; epilogue += x
            ptc = pt_c[ci]
            for nt in range(NT):
                if mid is not None and nt == NT - 2:
                    mid()
                ops = psum_g.tile([128, 512], F32, tag="g")
                for bj in range(CB):
                    nc.tensor.matmul(
                        ops[:],
                        ptc[:, 128 * bj : 128 * (bj + 1)],
                        nat[bj][:, 512 * nt : 512 * (nt + 1)],
                        start=(bj == 0),
                        stop=(bj == CB - 1),
                    )
                o_sb = outs_pool.tile([128, 512], F32, tag="o")
                nc.vector.scalar_tensor_tensor(
                    out=o_sb[:],
                    in0=ops[:],
                    scalar=gs_c[ci][:],
                    in1=nat[ci][:, 512 * nt : 512 * (nt + 1)].bitcast(F32),
                    op0=mybir.AluOpType.mult,
                    op1=mybir.AluOpType.add,
                )
                nc.gpsimd.dma_start(
                    out=out[
                        s, 128 * ci : 128 * (ci + 1), 512 * nt : 512 * (nt + 1)
                    ],
                    in_=o_sb[:],
                )

        for ci in range(CB):
            e = e_ps[ci]
            # stash SBUF copies of the blocks later row-panels will mirror
            for cj in range(ci + 1, CB):
                blk = eblk_pool.tile(
                    [128, 128], F32, tag="eblk", name=f"eblk{s}_{ci}_{cj}"
                )
                nc.vector.tensor_copy(
                    out=blk[:], in_=e[:, 128 * cj : 128 * (cj + 1)]
                )
                e_blk[(ci, cj)] = blk
            # mirror missing lower blocks from earlier panels
            for cj in range(ci):
                nc.tensor.transpose(
                    e[:, 128 * cj : 128 * (cj + 1)],
                    e_blk[(cj, ci)][:],
                    identity[:],
                )
            # softmax pieces: P = exp(m - e), S = rowsum, D = diag(gamma/S)
            m = small.tile([128, 1], F32, tag="m")
            nc.vector.tensor_reduce(
                out=m[:], in_=e[:], axis=mybir.AxisListType.X,
                op=mybir.AluOpType.min,
            )
            p = p_pool.tile([128, C], F16, tag="p")
            ssum = small.tile([128, 1], F32, tag="s")
            nc.scalar.activation(
                out=p[:], in_=e[:],
                func=mybir.ActivationFunctionType.Exp,
                bias=m[:], scale=-1.0, accum_out=ssum[:],
            )
            # gs = gamma / S feeds the mm2 epilogue as a per-row scale -
            # off the critical path (PT only needs p)
            r = small.tile([128, 1], F32, tag="r")
            nc.vector.reciprocal(out=r[:], in_=ssum[:])
            gs = small.tile([128, 1], F32, tag="gs")
            nc.vector.tensor_mul(out=gs[:], in0=r[:], in1=g_sb[:])
            gs_c.append(gs)

            # PT(ci) = P[ci].T @ D[ci]: [j, i] = gamma * att[i, j] for
            # i in block ci, laid out bj-grouped along the free axis.
            # Emitted near the end of mm2_block(ci-1) so the PSUM->SBUF
            # evacuation hides under the previous block's last matmuls.
            def emit_pt(p=p, ci=ci):
                ptp = psum_g.tile([128, C], F16, tag="g", name=f"ptp{s}_{ci}")
                for bj in range(CB):
                    nc.tensor.transpose(
                        ptp[:, 128 * bj : 128 * (bj + 1)],
                        p[:, 128 * bj : 128 * (bj + 1)],
                        id16[:],
                    )
                ptc = pt_pool.tile([128, C], F32R, tag="pt", name=f"ptc{s}_{ci}")
                nc.scalar.activation(
                    out=ptc[:], in_=ptp[:],
                    func=mybir.ActivationFunctionType.Copy,
                    bias=0.0, scale=1.0,
                )
                pt_c.append(ptc)

            # overlap: previous output block's mm2 runs while this
            # panel's softmax is still on DVE/ACT
            if ci > 0:
                mm2_block(ci - 1, mid=emit_pt)
            else:
                emit_pt()
                if s + 1 < BPC:
                    # next sample's load streams in during the mm2 blocks
                    nats[s + 1] = load_sample(s + 1)

        mm2_block(CB - 1)


_NC_CACHE = None


def _build():
    global _NC_CACHE
    if _NC_CACHE is not None:
        return _NC_CACHE
    from contextlib import ExitStack

    nc = bacc.Bacc("TRN2", target_bir_lowering=False)
    x = nc.dram_tensor("x", [BPC, C, N], F32, kind="ExternalInput")
    gamma = nc.dram_tensor("gamma", [1, 1], F32, kind="ExternalInput")
    out = nc.dram_tensor("out", [BPC, C, N], F32, kind="ExternalOutput")
    with tile.TileContext(nc) as tc:
        with ExitStack() as ctx:
            _emit(nc, tc, ctx, x[:], gamma[:], out[:])
    nc.compile()
    _NC_CACHE = nc
    return nc


def kernel(x, gamma):
    x = np.ascontiguousarray(np.asarray(x, dtype=np.float32))
    gamma = np.ascontiguousarray(np.asarray(gamma, dtype=np.float32))
    assert x.shape == (B, C, H, W), x.shape
    xf = x.reshape(B, C, N)
    nc = _build()
    in_maps = [
        {
            "x": xf[c * BPC : (c + 1) * BPC],
            "gamma": gamma.reshape(1, 1),
        }
        for c in range(NCORES)
    ]
    res = run_bass_kernel_spmd(nc, in_maps, core_ids=list(range(NCORES)))
    out = np.concatenate([res.results[c]["out"] for c in range(NCORES)], axis=0)
    return out.reshape(B, C, H, W)


# revision 11
# speedup vs baseline: 1.0955x; 1.0480x over previous
"""CAM (channel attention) module kernel for Trainium2, 8 NeuronCores.

Reference computation (per sample, x: [C, N] with C=512, N=64*64):
    energy    = x @ x.T                      # [C, C] symmetric Gram matrix
    energy_n  = rowmax(energy) - energy
    att       = softmax(energy_n, axis=-1)
    out       = gamma * (att @ x) + x

Softmax shift-invariance: softmax(rowmax - e) == softmax(-e), stabilized
with the row-min m_i:  att[i,j] = exp(m_i - e_ij) / S_i,  S_i = sum_j.

Sharding: pure data parallel over batch B=16 -> 2 samples per core.

Precision: the Gram matrix is computed from fp16 operands (10 mantissa
bits; ~1e-2 relative vs float64 at gamma=1), mm2 runs in fp32r; all
accumulation is fp32 PSUM and the "+ x" epilogue is exact fp32, so
gamma=0 reproduces x bit-exactly. fp16 runs the PE at 1 cycle/row for
ANY moving width (fp32r pays 4x below 256 wide), which makes the exact
128-block triangle + 128-wide PT matmuls cheap. The natural-layout
tiles are DECLARED f32r (bit-identical to f32), so no rounding pass is
needed anywhere: f32 -> fp16 rounding happens for free inside the
transpose-PSUM evacuations, and the PE truncates f32r mantissas itself.

Per-core pipeline (2 samples):
  1. load xf natural in 8 interleaved column pieces on the sync queue
     (output stores go on the otherwise-idle GpSimd queue so the next
     sample's loads are never stuck behind them); warmup matmuls keep
     the PE clock un-throttled (HAM) while the first pieces land
  2. per 128-col chunk k: PE-transpose the 4 channel blocks (f32r,
     1.5 cyc/row) -> PSUM, evacuate to fp16 xt (alternating ACT/DVE),
     then advance ALL four triangular Gram panels by one k step
     (wavefront) so the PE never queues idle work behind DMA pacing
  3. energy is symmetric: row panel ci computes columns [128*ci : 512]
     (exact upper triangle); lower blocks are mirrored from finished
     panels via PE transposes of stashed SBUF copies
  4. software-pipelined tail: for each ci - softmax(ci) [rowmin on DVE,
     P16 = exp(m - e) with fused row-sum on ACT, D = diag(gamma/S) in
     one fused DVE tensor_scalar], then mm2 block ci-1, then
     PT(ci) = P[ci].T @ D[ci] (4 fp16 128-wide matmuls, evacuated f32r
     in bj-grouped layout). mm2 for output block ci only needs
     softmax(ci), so each softmax hides under the previous mm2 block;
     only softmax(0)'s ~2.5us latency is exposed per sample
  5. mm2 block ci: out[ci] = sum_bj PT_ci[bj].T @ nat[bj] (moving
     operand is the f32r nat - no copies), epilogue out = psum + x in
     one DVE scalar_tensor_tensor pass (x read as exact fp32 bits)
  6. the next sample's FULL load is emitted before mm2 so it streams in
     during the ~14us of mm2 blocks and the boundary never stalls
"""

import numpy as np

import concourse.bacc as bacc
import concourse.tile as tile
from concourse import mybir
from concourse.bass_utils import run_bass_kernel_spmd
from concourse.masks import make_identity

B, C, H, W = 16, 512, 64, 64
N = H * W
NCORES = 8
BPC = B // NCORES  # samples per core
CB = C // 128      # channel blocks (4)
NK = N // 128      # 128-wide n-chunks (32)
NT = N // 512      # 512-wide n-tiles (8)

F32 = mybir.dt.float32
F32R = mybir.dt.float32r
F16 = mybir.dt.float16


def _emit(nc, tc, ctx, x, gamma, out):
    consts = ctx.enter_context(tc.tile_pool(name="consts", bufs=1))
    nat_pool = ctx.enter_context(tc.tile_pool(name="nat", bufs=2 * CB))
    xfT_pool = ctx.enter_context(tc.tile_pool(name="xfT", bufs=NK))
    p_pool = ctx.enter_context(tc.tile_pool(name="p", bufs=CB))
    pt_pool = ctx.enter_context(tc.tile_pool(name="pt", bufs=CB))
    eblk_pool = ctx.enter_context(tc.tile_pool(name="eblk", bufs=6))
    small = ctx.enter_context(tc.tile_pool(name="small", bufs=4 * CB))
    outs_pool = ctx.enter_context(tc.tile_pool(name="outs", bufs=4))
    psum_e = ctx.enter_context(tc.tile_pool(name="psum_e", bufs=CB, space="PSUM"))
    psum_g = ctx.enter_context(tc.tile_pool(name="psum_g", bufs=4, space="PSUM"))

    identity = consts.tile([128, 128], F32)
    make_identity(nc, identity[:])
    idR_t = consts.tile([128, 128], F32R)
    nc.vector.tensor_copy(out=idR_t[:], in_=identity[:])
    idR = idR_t[:]
    id16 = consts.tile([128, 128], F16)
    nc.vector.tensor_copy(out=id16[:], in_=identity[:])
    g_sb = consts.tile([128, 1], F32)
    nc.gpsimd.dma_start(out=g_sb[:], in_=gamma[:].to_broadcast((128, 1)))

    QN = N // 8

    def load_sample(s):
        nat = [
            nat_pool.tile([128, N], F32R, tag="nat", name=f"nat{s}_{c}")
            for c in range(CB)
        ]
        for q in range(8):
            for c in range(CB):
                nc.sync.dma_start(
                    out=nat[c][:, QN * q : QN * (q + 1)],
                    in_=x[
                        s, 128 * c : 128 * (c + 1), QN * q : QN * (q + 1)
                    ].bitcast(F32R),
                )
        return nat

    nats = {0: load_sample(0)}
    for s in range(BPC):
        nat = nats.pop(s)

        if s == 0:
            # keep the PE busy (HAM warm) while the first load pieces land
            warm_ps = psum_g.tile([128, 128], F32, tag="g", name=f"warm{s}")
            for w in range(28):
                nc.tensor.matmul(warm_ps[:], id16[:], id16[:], start=(w == 0), stop=False)
            nc.tensor.matmul(warm_ps[:], id16[:], id16[:], start=False, stop=True)

        # ---- transpose + triangular Gram wavefront: per k-chunk, PE
        # transposes the 4 blocks (f32r), the chunk is evacuated to fp16,
        # then ALL four row panels advance one accumulation step ----
        # panel ci covers columns [128*ci : 512] (upper triangle incl diag)
        e_ps = [
            psum_e.tile([128, C], F32, tag="e", name=f"e_ps{s}_{ci}")
            for ci in range(CB)
        ]
        # superchunks: transpose 8 chunks as one burst, then run their 32
        # panel matmuls back-to-back - the dense regular-matmul burst keeps
        # the HAM clock gate open (transpose-mode does not count as PE-busy)
        SCK = 8
        for sc in range(NK // SCK):
            xts = []
            for kk in range(SCK):
                k = SCK * sc + kk
                t_ps = psum_g.tile([128, C], F32R, tag="g")
                for c in range(CB):
                    nc.tensor.transpose(
                        t_ps[:, 128 * c : 128 * (c + 1)],
                        nat[c][:, 128 * k : 128 * (k + 1)],
                        idR,
                    )
                xt = xfT_pool.tile([128, C], F16, tag="xfT")
                if k % 2 == 0:
                    nc.scalar.activation(
                        out=xt[:], in_=t_ps[:].bitcast(F32),
                        func=mybir.ActivationFunctionType.Copy,
                        bias=0.0, scale=1.0,
                    )
                else:
                    nc.vector.tensor_copy(out=xt[:], in_=t_ps[:].bitcast(F32))
                xts.append(xt)
            # panel-major: panel 0 completes early in the final burst so
            # its softmax hides under the remaining panels' matmuls
            for ci in range(CB):
                lo = 128 * ci
                for kk, xt in enumerate(xts):
                    k = SCK * sc + kk
                    nc.tensor.matmul(
                        e_ps[ci][:, lo:C],
                        xt[:, lo : lo + 128],
                        xt[:, lo:C],
                        start=(k == 0),
                        stop=(k == NK - 1),
                    )

        # ---- software-pipelined softmax(ci) / mm2(ci-1) / PT(ci) ----
        e_blk = {}  # (ci, cj) -> SBUF f32 copy of energy block for mirroring
        pt_c = []   # per ci: PT columns 128*ci..128*(ci+1), bj-grouped
        gs_c = []   # per ci: gamma/S row scales for the epilogue

        def mm2_block(ci, early=None, mid=None):
            # out[ci] = sum_bj PT_ci[bj].T @ nat[bj] ; epilogue += x
            ptc = pt_c[ci]
            for nt in range(NT):
                if early is not None and nt == 2:
                    early()
                if mid is not None and nt == NT - 2:
                    mid()
                ops = psum_g.tile([128, 512], F32, tag="g")
                for bj in range(CB):
                    nc.tensor.matmul(
                        ops[:],
                        ptc[:, 128 * bj : 128 * (bj + 1)],
                        nat[bj][:, 512 * nt : 512 * (nt + 1)],
                        start=(bj == 0),
                        stop=(bj == CB - 1),
                    )
                o_sb = outs_pool.tile([128, 512], F32, tag="o")
                nc.vector.scalar_tensor_tensor(
                    out=o_sb[:],
                    in0=ops[:],
                    scalar=gs_c[ci][:],
                    in1=nat[ci][:, 512 * nt : 512 * (nt + 1)].bitcast(F32),
                    op0=mybir.AluOpType.mult,
                    op1=mybir.AluOpType.add,
                )
                st_q = nc.sync if (s + 1 >= BPC and nt % 2) else nc.gpsimd
                st_q.dma_start(
                    out=out[
                        s, 128 * ci : 128 * (ci + 1), 512 * nt : 512 * (nt + 1)
                    ],
                    in_=o_sb[:],
                )

        p_c = []

        def sm_block(ci):
            e = e_ps[ci]
            # stash SBUF copies of the blocks later row-panels will mirror
            for cj in range(ci + 1, CB):
                blk = eblk_pool.tile(
                    [128, 128], F32, tag="eblk", name=f"eblk{s}_{ci}_{cj}"
                )
                nc.vector.tensor_copy(
                    out=blk[:], in_=e[:, 128 * cj : 128 * (cj + 1)]
                )
                e_blk[(ci, cj)] = blk
            # mirror missing lower blocks from earlier panels
            for cj in range(ci):
                nc.tensor.transpose(
                    e[:, 128 * cj : 128 * (cj + 1)],
                    e_blk[(cj, ci)][:],
                    identity[:],
                )
            # softmax pieces: P = exp(m - e), S = rowsum
            m = small.tile([128, 1], F32, tag="m")
            nc.vector.tensor_reduce(
                out=m[:], in_=e[:], axis=mybir.AxisListType.X,
                op=mybir.AluOpType.min,
            )
            p = p_pool.tile([128, C], F16, tag="p")
            ssum = small.tile([128, 1], F32, tag="s")
            nc.scalar.activation(
                out=p[:], in_=e[:],
                func=mybir.ActivationFunctionType.Exp,
                bias=m[:], scale=-1.0, accum_out=ssum[:],
            )
            # gs = gamma / S feeds the mm2 epilogue as a per-row scale -
            # off the critical path (PT only needs p)
            r = small.tile([128, 1], F32, tag="r")
            nc.vector.reciprocal(out=r[:], in_=ssum[:])
            gs = small.tile([128, 1], F32, tag="gs")
            nc.vector.tensor_mul(out=gs[:], in0=r[:], in1=g_sb[:])
            gs_c.append(gs)
            p_c.append(p)

        def emit_pt(ci):
            # PT(ci) = P[ci].T: [j, i] = unnormalized att[i, j] for i in
            # block ci, laid out bj-grouped along the free axis (plain fp16
            # PE transposes; normalization + gamma fold into the epilogue)
            ptp = psum_g.tile([128, C], F16, tag="g", name=f"ptp{s}_{ci}")
            p = p_c[ci]
            for bj in range(CB):
                nc.tensor.transpose(
                    ptp[:, 128 * bj : 128 * (bj + 1)],
                    p[:, 128 * bj : 128 * (bj + 1)],
                    id16[:],
                )
            ptc = pt_pool.tile([128, C], F32R, tag="pt", name=f"ptc{s}_{ci}")
            nc.scalar.activation(
                out=ptc[:], in_=ptp[:],
                func=mybir.ActivationFunctionType.Copy,
                bias=0.0, scale=1.0,
            )
            pt_c.append(ptc)

        sm_block(0)
        emit_pt(0)
        if s + 1 < BPC:
            # next sample's full load streams in during the mm2 blocks
            nats[s + 1] = load_sample(s + 1)
        for ci in range(1, CB):
            mm2_block(
                ci - 1,
                early=lambda ci=ci: sm_block(ci),
                mid=lambda ci=ci: emit_pt(ci),
            )
        mm2_block(CB - 1)


_NC_CACHE = None


def _build():
    global _NC_CACHE
    if _NC_CACHE is not None:
        return _NC_CACHE
    from contextlib import ExitStack

    nc = bacc.Bacc("TRN2", target_bir_lowering=False)
    x = nc.dram_tensor("x", [BPC, C, N], F32, kind="ExternalInput")
    gamma = nc.dram_tensor("gamma", [1, 1], F32, kind="ExternalInput")
    out = nc.dram_tensor("out", [BPC, C, N], F32, kind="ExternalOutput")
    with tile.TileContext(nc) as tc:
        with ExitStack() as ctx:
            _emit(nc, tc, ctx, x[:], gamma[:], out[:])
    nc.compile()
    _NC_CACHE = nc
    return nc


def kernel(x, gamma):
    x = np.ascontiguousarray(np.asarray(x, dtype=np.float32))
    gamma = np.ascontiguousarray(np.asarray(gamma, dtype=np.float32))
    assert x.shape == (B, C, H, W), x.shape
    xf = x.reshape(B, C, N)
    nc = _build()
    in_maps = [
        {
            "x": xf[c * BPC : (c + 1) * BPC],
            "gamma": gamma.reshape(1, 1),
        }
        for c in range(NCORES)
    ]
    res = run_bass_kernel_spmd(nc, in_maps, core_ids=list(range(NCORES)))
    out = np.concatenate([res.results[c]["out"] for c in range(NCORES)], axis=0)
    return out.reshape(B, C, H, W)


# revision 12
# speedup vs baseline: 1.1102x; 1.0134x over previous
"""CAM (channel attention) module kernel for Trainium2, 8 NeuronCores.

Reference computation (per sample, x: [C, N] with C=512, N=64*64):
    energy    = x @ x.T                      # [C, C] symmetric Gram matrix
    energy_n  = rowmax(energy) - energy
    att       = softmax(energy_n, axis=-1)
    out       = gamma * (att @ x) + x

Softmax shift-invariance: softmax(rowmax - e) == softmax(-e), stabilized
with the row-min m_i:  att[i,j] = exp(m_i - e_ij) / S_i,  S_i = sum_j.

Sharding: pure data parallel over batch B=16 -> 2 samples per core.

Precision: the Gram matrix is computed from fp16 operands (10 mantissa
bits; ~1e-2 relative vs float64 at gamma=1), mm2 runs in fp32r; all
accumulation is fp32 PSUM and the "+ x" epilogue is exact fp32, so
gamma=0 reproduces x bit-exactly. fp16 runs the PE at 1 cycle/row for
ANY moving width (fp32r pays 4x below 256 wide), which makes the exact
128-block triangle + 128-wide PT matmuls cheap. The natural-layout
tiles are DECLARED f32r (bit-identical to f32), so no rounding pass is
needed anywhere: f32 -> fp16 rounding happens for free inside the
transpose-PSUM evacuations, and the PE truncates f32r mantissas itself.

Per-core pipeline (2 samples):
  1. load xf natural in 8 interleaved column pieces on the sync queue
     (output stores go on the otherwise-idle GpSimd queue so the next
     sample's loads are never stuck behind them); warmup matmuls keep
     the PE clock un-throttled (HAM) while the first pieces land
  2. per 128-col chunk k: PE-transpose the 4 channel blocks (f32r,
     1.5 cyc/row) -> PSUM, evacuate to fp16 xt (alternating ACT/DVE),
     then advance ALL four triangular Gram panels by one k step
     (wavefront) so the PE never queues idle work behind DMA pacing
  3. energy is symmetric: row panel ci computes columns [128*ci : 512]
     (exact upper triangle); lower blocks are mirrored from finished
     panels via PE transposes of stashed SBUF copies
  4. software-pipelined tail: for each ci - softmax(ci) [rowmin on DVE,
     P16 = exp(m - e) with fused row-sum on ACT, D = diag(gamma/S) in
     one fused DVE tensor_scalar], then mm2 block ci-1, then
     PT(ci) = P[ci].T @ D[ci] (4 fp16 128-wide matmuls, evacuated f32r
     in bj-grouped layout). mm2 for output block ci only needs
     softmax(ci), so each softmax hides under the previous mm2 block;
     only softmax(0)'s ~2.5us latency is exposed per sample
  5. mm2 block ci: out[ci] = sum_bj PT_ci[bj].T @ nat[bj] (moving
     operand is the f32r nat - no copies), epilogue out = psum + x in
     one DVE scalar_tensor_tensor pass (x read as exact fp32 bits)
  6. the next sample's FULL load is emitted before mm2 so it streams in
     during the ~14us of mm2 blocks and the boundary never stalls
"""

import numpy as np

import concourse.bacc as bacc
import concourse.tile as tile
from concourse import mybir
from concourse.bass_utils import run_bass_kernel_spmd
from concourse.masks import make_identity

B, C, H, W = 16, 512, 64, 64
N = H * W
NCORES = 8
BPC = B // NCORES  # samples per core
CB = C // 128      # channel blocks (4)
NK = N // 128      # 128-wide n-chunks (32)
NT = N // 512      # 512-wide n-tiles (8)

F32 = mybir.dt.float32
F32R = mybir.dt.float32r
F16 = mybir.dt.float16


def _emit(nc, tc, ctx, x, gamma, out):
    consts = ctx.enter_context(tc.tile_pool(name="consts", bufs=1))
    nat_pool = ctx.enter_context(tc.tile_pool(name="nat", bufs=2 * CB))
    xfT_pool = ctx.enter_context(tc.tile_pool(name="xfT", bufs=NK))
    p_pool = ctx.enter_context(tc.tile_pool(name="p", bufs=CB))
    pt_pool = ctx.enter_context(tc.tile_pool(name="pt", bufs=CB))
    eblk_pool = ctx.enter_context(tc.tile_pool(name="eblk", bufs=6))
    small = ctx.enter_context(tc.tile_pool(name="small", bufs=4 * CB))
    outs_pool = ctx.enter_context(tc.tile_pool(name="outs", bufs=4))
    psum_e = ctx.enter_context(tc.tile_pool(name="psum_e", bufs=CB, space="PSUM"))
    psum_g = ctx.enter_context(tc.tile_pool(name="psum_g", bufs=4, space="PSUM"))

    identity = consts.tile([128, 128], F32)
    make_identity(nc, identity[:])
    idR_t = consts.tile([128, 128], F32R)
    nc.vector.tensor_copy(out=idR_t[:], in_=identity[:])
    idR = idR_t[:]
    id16 = consts.tile([128, 128], F16)
    nc.vector.tensor_copy(out=id16[:], in_=identity[:])
    g_sb = consts.tile([128, 1], F32)
    nc.gpsimd.dma_start(out=g_sb[:], in_=gamma[:].to_broadcast((128, 1)))

    QN = N // 8

    def load_sample(s):
        nat = [
            nat_pool.tile([128, N], F32R, tag="nat", name=f"nat{s}_{c}")
            for c in range(CB)
        ]
        for q in range(8):
            for c in range(CB):
                nc.sync.dma_start(
                    out=nat[c][:, QN * q : QN * (q + 1)],
                    in_=x[
                        s, 128 * c : 128 * (c + 1), QN * q : QN * (q + 1)
                    ].bitcast(F32R),
                )
        return nat

    nats = {0: load_sample(0)}
    for s in range(BPC):
        nat = nats.pop(s)

        if s == 0:
            # keep the PE busy (HAM warm) while the first load pieces land
            warm_ps = psum_g.tile([128, 128], F32, tag="g", name=f"warm{s}")
            for w in range(28):
                nc.tensor.matmul(warm_ps[:], id16[:], id16[:], start=(w == 0), stop=False)
            nc.tensor.matmul(warm_ps[:], id16[:], id16[:], start=False, stop=True)

        # ---- transpose + triangular Gram wavefront: per k-chunk, PE
        # transposes the 4 blocks (f32r), the chunk is evacuated to fp16,
        # then ALL four row panels advance one accumulation step ----
        # panel ci covers columns [128*ci : 512] (upper triangle incl diag)
        e_ps = [
            psum_e.tile([128, C], F32, tag="e", name=f"e_ps{s}_{ci}")
            for ci in range(CB)
        ]
        # superchunks: transpose 8 chunks as one burst, then run their 32
        # panel matmuls back-to-back - the dense regular-matmul burst keeps
        # the HAM clock gate open (transpose-mode does not count as PE-busy)
        SCK = 8
        for sc in range(NK // SCK):
            xts = []
            for kk in range(SCK):
                k = SCK * sc + kk
                t_ps = psum_g.tile([128, C], F32R, tag="g")
                for c in range(CB):
                    nc.tensor.transpose(
                        t_ps[:, 128 * c : 128 * (c + 1)],
                        nat[c][:, 128 * k : 128 * (k + 1)],
                        idR,
                    )
                xt = xfT_pool.tile([128, C], F16, tag="xfT")
                if k % 2 == 0:
                    nc.scalar.activation(
                        out=xt[:], in_=t_ps[:].bitcast(F32),
                        func=mybir.ActivationFunctionType.Copy,
                        bias=0.0, scale=1.0,
                    )
                else:
                    nc.vector.tensor_copy(out=xt[:], in_=t_ps[:].bitcast(F32))
                xts.append(xt)
            # panel-major: panel 0 completes early in the final burst so
            # its softmax hides under the remaining panels' matmuls
            for ci in range(CB):
                lo = 128 * ci
                for kk, xt in enumerate(xts):
                    k = SCK * sc + kk
                    nc.tensor.matmul(
                        e_ps[ci][:, lo:C],
                        xt[:, lo : lo + 128],
                        xt[:, lo:C],
                        start=(k == 0),
                        stop=(k == NK - 1),
                    )

        # ---- software-pipelined softmax(ci) / mm2(ci-1) / PT(ci) ----
        e_blk = {}  # (ci, cj) -> SBUF f32 copy of energy block for mirroring
        pt_c = {}   # per ci: PT columns 128*ci..128*(ci+1), bj-grouped
        gs_c = {}   # per ci: gamma/S row scales for the epilogue
        p_c = {}    # per ci: P16 = exp(m - e)
        ss_c = {}   # per ci: S row sums

        def sm_pre(ci):
            # stash, mirrors, rowmin, exp: the latency-laden ACT round-trip
            # starts here; nothing below depends on it until sm_post
            e = e_ps[ci]
            for cj in range(ci + 1, CB):
                blk = eblk_pool.tile(
                    [128, 128], F32, tag="eblk", name=f"eblk{s}_{ci}_{cj}"
                )
                nc.vector.tensor_copy(
                    out=blk[:], in_=e[:, 128 * cj : 128 * (cj + 1)]
                )
                e_blk[(ci, cj)] = blk
            for cj in range(ci):
                nc.tensor.transpose(
                    e[:, 128 * cj : 128 * (cj + 1)],
                    e_blk[(cj, ci)][:],
                    identity[:],
                )
            m = small.tile([128, 1], F32, tag="m")
            nc.vector.tensor_reduce(
                out=m[:], in_=e[:], axis=mybir.AxisListType.X,
                op=mybir.AluOpType.min,
            )
            p = p_pool.tile([128, C], F16, tag="p")
            ssum = small.tile([128, 1], F32, tag="s")
            nc.scalar.activation(
                out=p[:], in_=e[:],
                func=mybir.ActivationFunctionType.Exp,
                bias=m[:], scale=-1.0, accum_out=ssum[:],
            )
            p_c[ci] = p
            ss_c[ci] = ssum

        def sm_post(ci):
            # gs = gamma / S feeds the mm2 epilogue as a per-row scale
            r = small.tile([128, 1], F32, tag="r")
            nc.vector.reciprocal(out=r[:], in_=ss_c[ci][:])
            gs = small.tile([128, 1], F32, tag="gs")
            nc.vector.tensor_mul(out=gs[:], in0=r[:], in1=g_sb[:])
            gs_c[ci] = gs

        def emit_pt(ci):
            # PT(ci) = P[ci].T: [j, i] = unnormalized att[i, j] for i in
            # block ci, bj-grouped along the free axis (plain fp16 PE
            # transposes; normalization + gamma fold into the epilogue)
            ptp = psum_g.tile([128, C], F16, tag="g", name=f"ptp{s}_{ci}")
            p = p_c[ci]
            for bj in range(CB):
                nc.tensor.transpose(
                    ptp[:, 128 * bj : 128 * (bj + 1)],
                    p[:, 128 * bj : 128 * (bj + 1)],
                    id16[:],
                )
            ptc = pt_pool.tile([128, C], F32R, tag="pt", name=f"ptc{s}_{ci}")
            nc.scalar.activation(
                out=ptc[:], in_=ptp[:],
                func=mybir.ActivationFunctionType.Copy,
                bias=0.0, scale=1.0,
            )
            pt_c[ci] = ptc

        def mm2_group(ci, nt):
            # out[ci, nt] = (sum_bj PT_ci[bj].T @ nat[bj]) * gs + x
            ops = psum_g.tile([128, 512], F32, tag="g", name=f"ops{s}_{ci}_{nt}")
            for bj in range(CB):
                nc.tensor.matmul(
                    ops[:],
                    pt_c[ci][:, 128 * bj : 128 * (bj + 1)],
                    nat[bj][:, 512 * nt : 512 * (nt + 1)],
                    start=(bj == 0),
                    stop=(bj == CB - 1),
                )
            o_sb = outs_pool.tile([128, 512], F32, tag="o")
            nc.vector.scalar_tensor_tensor(
                out=o_sb[:],
                in0=ops[:],
                scalar=gs_c[ci][:],
                in1=nat[ci][:, 512 * nt : 512 * (nt + 1)].bitcast(F32),
                op0=mybir.AluOpType.mult,
                op1=mybir.AluOpType.add,
            )
            st_q = nc.sync if (s + 1 >= BPC and nt % 2) else nc.gpsimd
            st_q.dma_start(
                out=out[
                    s, 128 * ci : 128 * (ci + 1), 512 * nt : 512 * (nt + 1)
                ],
                in_=o_sb[:],
            )

        def mm2_block(ci, early=None, mid=None):
            for nt in range(NT):
                if early is not None and nt == 2:
                    early()
                if mid is not None and nt == NT - 2:
                    mid()
                mm2_group(ci, nt)

        sm_pre(0)
        sm_post(0)
        emit_pt(0)
        if s + 1 < BPC:
            # next sample's full load streams in during the mm2 blocks
            nats[s + 1] = load_sample(s + 1)
            mm2_block(
                0,
                early=lambda: sm_pre(1),
                mid=lambda: (sm_post(1), emit_pt(1)),
            )
            mm2_block(
                1,
                early=lambda: sm_pre(2),
                mid=lambda: (sm_post(2), emit_pt(2)),
            )
            mm2_block(
                2,
                early=lambda: sm_pre(3),
                mid=lambda: (sm_post(3), emit_pt(3)),
            )
            mm2_block(3)
        else:
            # last sample: pull softmax(3) forward and interleave the final
            # two output blocks so the output DMA is not bunched at the end
            mm2_block(
                0,
                early=lambda: sm_pre(1),
                mid=lambda: (sm_post(1), emit_pt(1)),
            )
            mm2_block(
                1,
                early=lambda: sm_pre(2),
                mid=lambda: (sm_post(2), emit_pt(2), sm_pre(3)),
            )
            sm_post(3)
            emit_pt(3)
            for nt in range(NT):
                mm2_group(2, nt)
                mm2_group(3, nt)


_NC_CACHE = None


def _build():
    global _NC_CACHE
    if _NC_CACHE is not None:
        return _NC_CACHE
    from contextlib import ExitStack

    nc = bacc.Bacc("TRN2", target_bir_lowering=False)
    x = nc.dram_tensor("x", [BPC, C, N], F32, kind="ExternalInput")
    gamma = nc.dram_tensor("gamma", [1, 1], F32, kind="ExternalInput")
    out = nc.dram_tensor("out", [BPC, C, N], F32, kind="ExternalOutput")
    with tile.TileContext(nc) as tc:
        with ExitStack() as ctx:
            _emit(nc, tc, ctx, x[:], gamma[:], out[:])
    nc.compile()
    _NC_CACHE = nc
    return nc


def kernel(x, gamma):
    x = np.ascontiguousarray(np.asarray(x, dtype=np.float32))
    gamma = np.ascontiguousarray(np.asarray(gamma, dtype=np.float32))
    assert x.shape == (B, C, H, W), x.shape
    xf = x.reshape(B, C, N)
    nc = _build()
    in_maps = [
        {
            "x": xf[c * BPC : (c + 1) * BPC],
            "gamma": gamma.reshape(1, 1),
        }
        for c in range(NCORES)
    ]
    res = run_bass_kernel_spmd(nc, in_maps, core_ids=list(range(NCORES)))
    out = np.concatenate([res.results[c]["out"] for c in range(NCORES)], axis=0)
    return out.reshape(B, C, H, W)


# revision 13
# speedup vs baseline: 1.1236x; 1.0121x over previous
"""CAM (channel attention) module kernel for Trainium2, 8 NeuronCores.

Reference computation (per sample, x: [C, N] with C=512, N=64*64):
    energy    = x @ x.T                      # [C, C] symmetric Gram matrix
    energy_n  = rowmax(energy) - energy
    att       = softmax(energy_n, axis=-1)
    out       = gamma * (att @ x) + x

Softmax shift-invariance: softmax(rowmax - e) == softmax(-e), stabilized
with the row-min m_i:  att[i,j] = exp(m_i - e_ij) / S_i,  S_i = sum_j.

Sharding: pure data parallel over batch B=16 -> 2 samples per core.

Precision: the Gram matrix is computed from fp16 operands (10 mantissa
bits; ~1e-2 relative vs float64 at gamma=1), mm2 runs in fp32r; all
accumulation is fp32 PSUM and the "+ x" epilogue is exact fp32, so
gamma=0 reproduces x bit-exactly. fp16 runs the PE at 1 cycle/row for
ANY moving width (fp32r pays 4x below 256 wide), which makes the exact
128-block triangle + 128-wide PT matmuls cheap. The natural-layout
tiles are DECLARED f32r (bit-identical to f32), so no rounding pass is
needed anywhere: f32 -> fp16 rounding happens for free inside the
transpose-PSUM evacuations, and the PE truncates f32r mantissas itself.

Per-core pipeline (2 samples):
  1. load xf natural in 8 interleaved column pieces on the sync queue
     (output stores go on the otherwise-idle GpSimd queue so the next
     sample's loads are never stuck behind them); warmup matmuls keep
     the PE clock un-throttled (HAM) while the first pieces land
  2. per 128-col chunk k: PE-transpose the 4 channel blocks (f32r,
     1.5 cyc/row) -> PSUM, evacuate to fp16 xt (alternating ACT/DVE),
     then advance ALL four triangular Gram panels by one k step
     (wavefront) so the PE never queues idle work behind DMA pacing
  3. energy is symmetric: row panel ci computes columns [128*ci : 512]
     (exact upper triangle); lower blocks are mirrored from finished
     panels via PE transposes of stashed SBUF copies
  4. software-pipelined tail: for each ci - softmax(ci) [rowmin on DVE,
     P16 = exp(m - e) with fused row-sum on ACT, D = diag(gamma/S) in
     one fused DVE tensor_scalar], then mm2 block ci-1, then
     PT(ci) = P[ci].T @ D[ci] (4 fp16 128-wide matmuls, evacuated f32r
     in bj-grouped layout). mm2 for output block ci only needs
     softmax(ci), so each softmax hides under the previous mm2 block;
     only softmax(0)'s ~2.5us latency is exposed per sample
  5. mm2 block ci: out[ci] = sum_bj PT_ci[bj].T @ nat[bj] (moving
     operand is the f32r nat - no copies), epilogue out = psum + x in
     one DVE scalar_tensor_tensor pass (x read as exact fp32 bits)
  6. the next sample's FULL load is emitted before mm2 so it streams in
     during the ~14us of mm2 blocks and the boundary never stalls
"""

import numpy as np

import concourse.bacc as bacc
import concourse.tile as tile
from concourse import mybir
from concourse.bass_utils import run_bass_kernel_spmd
from concourse.masks import make_identity

B, C, H, W = 16, 512, 64, 64
N = H * W
NCORES = 8
BPC = B // NCORES  # samples per core
CB = C // 128      # channel blocks (4)
NK = N // 128      # 128-wide n-chunks (32)
NT = N // 512      # 512-wide n-tiles (8)

F32 = mybir.dt.float32
F32R = mybir.dt.float32r
F16 = mybir.dt.float16


def _emit(nc, tc, ctx, x, gamma, out):
    consts = ctx.enter_context(tc.tile_pool(name="consts", bufs=1))
    nat_pool = ctx.enter_context(tc.tile_pool(name="nat", bufs=2 * CB))
    xfT_pool = ctx.enter_context(tc.tile_pool(name="xfT", bufs=NK))
    p_pool = ctx.enter_context(tc.tile_pool(name="p", bufs=CB))
    pt_pool = ctx.enter_context(tc.tile_pool(name="pt", bufs=CB))
    eblk_pool = ctx.enter_context(tc.tile_pool(name="eblk", bufs=6))
    small = ctx.enter_context(tc.tile_pool(name="small", bufs=4 * CB))
    outs_pool = ctx.enter_context(tc.tile_pool(name="outs", bufs=4))
    psum_e = ctx.enter_context(tc.tile_pool(name="psum_e", bufs=CB, space="PSUM"))
    psum_g = ctx.enter_context(tc.tile_pool(name="psum_g", bufs=4, space="PSUM"))

    identity = consts.tile([128, 128], F32)
    make_identity(nc, identity[:])
    idR_t = consts.tile([128, 128], F32R)
    nc.vector.tensor_copy(out=idR_t[:], in_=identity[:])
    idR = idR_t[:]
    id16 = consts.tile([128, 128], F16)
    nc.vector.tensor_copy(out=id16[:], in_=identity[:])
    g_sb = consts.tile([128, 1], F32)
    nc.gpsimd.dma_start(out=g_sb[:], in_=gamma[:].to_broadcast((128, 1)))

    QN = N // 8

    def load_sample(s):
        nat = [
            nat_pool.tile([128, N], F32R, tag="nat", name=f"nat{s}_{c}")
            for c in range(CB)
        ]
        for q in range(8):
            for c in range(CB):
                nc.sync.dma_start(
                    out=nat[c][:, QN * q : QN * (q + 1)],
                    in_=x[
                        s, 128 * c : 128 * (c + 1), QN * q : QN * (q + 1)
                    ].bitcast(F32R),
                )
        return nat

    nats = {0: load_sample(0)}
    for s in range(BPC):
        nat = nats.pop(s)

        if s == 0:
            # keep the PE busy (HAM warm) while the first load pieces land
            warm_ps = psum_g.tile([128, 128], F32, tag="g", name=f"warm{s}")
            for w in range(28):
                nc.tensor.matmul(warm_ps[:], id16[:], id16[:], start=(w == 0), stop=False)
            nc.tensor.matmul(warm_ps[:], id16[:], id16[:], start=False, stop=True)

        # ---- transpose + triangular Gram wavefront: per k-chunk, PE
        # transposes the 4 blocks (f32r), the chunk is evacuated to fp16,
        # then ALL four row panels advance one accumulation step ----
        # panel ci covers columns [128*ci : 512] (upper triangle incl diag)
        e_ps = [
            psum_e.tile([128, C], F32, tag="e", name=f"e_ps{s}_{ci}")
            for ci in range(CB)
        ]
        # superchunks: transpose 8 chunks as one burst, then run their 32
        # panel matmuls back-to-back - the dense regular-matmul burst keeps
        # the HAM clock gate open (transpose-mode does not count as PE-busy)
        SCK = 8
        for sc in range(NK // SCK):
            xts = []
            for kk in range(SCK):
                k = SCK * sc + kk
                t_ps = psum_g.tile([128, C], F32R, tag="g")
                for c in range(CB):
                    nc.tensor.transpose(
                        t_ps[:, 128 * c : 128 * (c + 1)],
                        nat[c][:, 128 * k : 128 * (k + 1)],
                        idR,
                    )
                xt = xfT_pool.tile([128, C], F16, tag="xfT")
                if k % 2 == 0:
                    nc.scalar.activation(
                        out=xt[:], in_=t_ps[:].bitcast(F32),
                        func=mybir.ActivationFunctionType.Copy,
                        bias=0.0, scale=1.0,
                    )
                else:
                    nc.vector.tensor_copy(out=xt[:], in_=t_ps[:].bitcast(F32))
                xts.append(xt)
            # panel-major: panel 0 completes early in the final burst so
            # its softmax hides under the remaining panels' matmuls
            for ci in range(CB):
                lo = 128 * ci
                for kk, xt in enumerate(xts):
                    k = SCK * sc + kk
                    nc.tensor.matmul(
                        e_ps[ci][:, lo:C],
                        xt[:, lo : lo + 128],
                        xt[:, lo:C],
                        start=(k == 0),
                        stop=(k == NK - 1),
                    )

        # ---- software-pipelined softmax(ci) / mm2(ci-1) / PT(ci) ----
        e_blk = {}  # (ci, cj) -> SBUF f32 copy of energy block for mirroring
        pt_c = {}   # per ci: PT columns 128*ci..128*(ci+1), bj-grouped
        gs_c = {}   # per ci: gamma/S row scales for the epilogue
        p_c = {}    # per ci: P16 = exp(m - e)
        ss_c = {}   # per ci: S row sums

        def sm_pre(ci):
            # stash, mirrors, rowmin, exp: the latency-laden ACT round-trip
            # starts here; nothing below depends on it until sm_post
            e = e_ps[ci]
            for cj in range(ci + 1, CB):
                blk = eblk_pool.tile(
                    [128, 128], F32, tag="eblk", name=f"eblk{s}_{ci}_{cj}"
                )
                nc.vector.tensor_copy(
                    out=blk[:], in_=e[:, 128 * cj : 128 * (cj + 1)]
                )
                e_blk[(ci, cj)] = blk
            for cj in range(ci):
                nc.tensor.transpose(
                    e[:, 128 * cj : 128 * (cj + 1)],
                    e_blk[(cj, ci)][:],
                    identity[:],
                )
            m = small.tile([128, 1], F32, tag="m")
            nc.vector.tensor_reduce(
                out=m[:], in_=e[:], axis=mybir.AxisListType.X,
                op=mybir.AluOpType.min,
            )
            p = p_pool.tile([128, C], F16, tag="p")
            ssum = small.tile([128, 1], F32, tag="s")
            nc.scalar.activation(
                out=p[:], in_=e[:],
                func=mybir.ActivationFunctionType.Exp,
                bias=m[:], scale=-1.0, accum_out=ssum[:],
            )
            p_c[ci] = p
            ss_c[ci] = ssum

        def sm_post(ci):
            # gs = gamma / S feeds the mm2 epilogue as a per-row scale
            r = small.tile([128, 1], F32, tag="r")
            nc.vector.reciprocal(out=r[:], in_=ss_c[ci][:])
            gs = small.tile([128, 1], F32, tag="gs")
            nc.vector.tensor_mul(out=gs[:], in0=r[:], in1=g_sb[:])
            gs_c[ci] = gs

        def emit_pt(ci):
            # PT(ci) = P[ci].T: [j, i] = unnormalized att[i, j] for i in
            # block ci, bj-grouped along the free axis (plain fp16 PE
            # transposes; normalization + gamma fold into the epilogue)
            ptp = psum_g.tile([128, C], F16, tag="g", name=f"ptp{s}_{ci}")
            p = p_c[ci]
            for bj in range(CB):
                nc.tensor.transpose(
                    ptp[:, 128 * bj : 128 * (bj + 1)],
                    p[:, 128 * bj : 128 * (bj + 1)],
                    id16[:],
                )
            ptc = pt_pool.tile([128, C], F32R, tag="pt", name=f"ptc{s}_{ci}")
            nc.scalar.activation(
                out=ptc[:], in_=ptp[:],
                func=mybir.ActivationFunctionType.Copy,
                bias=0.0, scale=1.0,
            )
            pt_c[ci] = ptc

        def mm2_group(ci, nt):
            # out[ci, nt] = (sum_bj PT_ci[bj].T @ nat[bj]) * gs + x
            ops = psum_g.tile([128, 512], F32, tag="g", name=f"ops{s}_{ci}_{nt}")
            for bj in range(CB):
                nc.tensor.matmul(
                    ops[:],
                    pt_c[ci][:, 128 * bj : 128 * (bj + 1)],
                    nat[bj][:, 512 * nt : 512 * (nt + 1)],
                    start=(bj == 0),
                    stop=(bj == CB - 1),
                )
            o_sb = outs_pool.tile([128, 512], F32, tag="o")
            nc.vector.scalar_tensor_tensor(
                out=o_sb[:],
                in0=ops[:],
                scalar=gs_c[ci][:],
                in1=nat[ci][:, 512 * nt : 512 * (nt + 1)].bitcast(F32),
                op0=mybir.AluOpType.mult,
                op1=mybir.AluOpType.add,
            )
            st_q = nc.sync if (s + 1 >= BPC and nt % 2) else nc.gpsimd
            st_q.dma_start(
                out=out[
                    s, 128 * ci : 128 * (ci + 1), 512 * nt : 512 * (nt + 1)
                ],
                in_=o_sb[:],
            )

        def mm2_block(ci, early=None, mid=None):
            for nt in range(NT):
                if early is not None and nt == 2:
                    early()
                if mid is not None and nt == NT - 2:
                    mid()
                mm2_group(ci, nt)

        # schedule: sm_pre(ci) runs TWO blocks ahead of mm2(ci) so the
        # rowmin->exp ACT round-trip never sits at the head of the DVE
        # queue when epilogue STTs (which recycle PSUM slots) are due
        sm_pre(0)
        sm_post(0)
        emit_pt(0)
        sm_pre(1)
        if s + 1 < BPC:
            # next sample's full load streams in during the mm2 blocks
            nats[s + 1] = load_sample(s + 1)
            mm2_block(0, mid=lambda: (sm_post(1), emit_pt(1), sm_pre(2)))
            mm2_block(1, mid=lambda: (sm_post(2), emit_pt(2), sm_pre(3)))
            mm2_block(2, mid=lambda: (sm_post(3), emit_pt(3)))
            mm2_block(3)
        else:
            # last sample: pull softmax(3) forward and interleave the final
            # two output blocks so the output DMA is not bunched at the end
            mm2_block(0, mid=lambda: (sm_post(1), emit_pt(1), sm_pre(2)))
            mm2_block(1, mid=lambda: (sm_post(2), emit_pt(2), sm_pre(3)))
            sm_post(3)
            emit_pt(3)
            for nt in range(NT):
                mm2_group(2, nt)
                mm2_group(3, nt)


_NC_CACHE = None


def _build():
    global _NC_CACHE
    if _NC_CACHE is not None:
        return _NC_CACHE
    from contextlib import ExitStack

    nc = bacc.Bacc("TRN2", target_bir_lowering=False)
    x = nc.dram_tensor("x", [BPC, C, N], F32, kind="ExternalInput")
    gamma = nc.dram_tensor("gamma", [1, 1], F32, kind="ExternalInput")
    out = nc.dram_tensor("out", [BPC, C, N], F32, kind="ExternalOutput")
    with tile.TileContext(nc) as tc:
        with ExitStack() as ctx:
            _emit(nc, tc, ctx, x[:], gamma[:], out[:])
    nc.compile()
    _NC_CACHE = nc
    return nc


def kernel(x, gamma):
    x = np.ascontiguousarray(np.asarray(x, dtype=np.float32))
    gamma = np.ascontiguousarray(np.asarray(gamma, dtype=np.float32))
    assert x.shape == (B, C, H, W), x.shape
    xf = x.reshape(B, C, N)
    nc = _build()
    in_maps = [
        {
            "x": xf[c * BPC : (c + 1) * BPC],
            "gamma": gamma.reshape(1, 1),
        }
        for c in range(NCORES)
    ]
    res = run_bass_kernel_spmd(nc, in_maps, core_ids=list(range(NCORES)))
    out = np.concatenate([res.results[c]["out"] for c in range(NCORES)], axis=0)
    return out.reshape(B, C, H, W)
